# revision 5
# baseline (speedup 1.0000x reference)
"""Trainium2 Bass kernel for CubicalLayer gather_nd.

Problem: X[4096,4096] f32, indices[524288,2] int32 ->
         out[262144,2] f32, out.flat[k] = X[indices[k,0], indices[k,1]].

Strategy (data-parallel over the pair list, 8 NeuronCores):
  - Host shards the pair list by X row-stripe (512 rows -> 8MB per core) so
    each core's gather window fits dma_gather's int16 index range.
  - Device (per core): computes int16 256B-block indices on DVE, bulk-gathers
    64-float blocks from HBM via the SWDGE dma_gather custom instruction
    (1024 indices per instruction, 4 SWDGE queues), then selects the target
    element from each block with an iota/one-hot compare + multiply + reduce
    on the vector engine. Everything is pipelined under Tile.
  - Host unshards: scatters per-core results back to original pair order.
"""

import numpy as np

import concourse.tile as tile
from concourse import bacc, mybir
from concourse.bass_utils import run_bass_kernel_spmd

H = 4096
W = 4096
N_IDX = 524288
NCORES = 8
P = 128

STRIPE_ROWS = H // NCORES  # 512
ELEM = 64  # f32 per gathered block (256B)
BLOCKS_PER_STRIPE = STRIPE_ROWS * (W // ELEM)  # 32768 (int16-indexable)

NPAD = 69632  # padded per-core pair count: 68*1024 = 17*4096
GCHUNK = 1024  # indices per dma_gather instruction (SWDGE ring capacity)
LCHUNK = 4096  # indices per pair-load chunk
NQ = 4  # SWDGE queues

F16 = NPAD // 16  # idx-tile free dim (4352)
COLS = NPAD // P  # out free dim (544)


def build_kernel(npad=NPAD, reps=1):
    f16 = npad // 16
    cols = npad // P
    n_lchunks = npad // LCHUNK
    gathers_per_l = LCHUNK // GCHUNK  # 4
    cg = GCHUNK // P  # groups per gather chunk (8)

    nc = bacc.Bacc(
        "TRN2",
        target_bir_lowering=False,
        debug=False,
        num_devices=NCORES,
        num_swdge_queues=NQ,
    )
    XS = nc.dram_tensor("XS", [STRIPE_ROWS, W], mybir.dt.float32, kind="ExternalInput")
    # wrapped pair list: pair k at [k%16 (replicated x8), k//16, {r_local, c}]
    pairs = nc.dram_tensor("pairs", [P, f16, 2], mybir.dt.int32, kind="ExternalInput")
    # c values in out-slot order: pair k at [k%128, k//128]
    cR = nc.dram_tensor("cR", [P, cols], mybir.dt.int32, kind="ExternalInput")
    out = nc.dram_tensor("out", [P, cols], mybir.dt.float32, kind="ExternalOutput")

    xs_rows = XS.ap().rearrange("h (a b) -> (h a) b", b=ELEM)  # [32768, 64]

    with tile.TileContext(nc) as tc:
        with (
            tc.tile_pool(name="const", bufs=1) as const_pool,
            tc.tile_pool(name="pairp", bufs=3) as pair_pool,
            tc.tile_pool(name="blkp", bufs=1) as blk_pool,
            tc.tile_pool(name="tmpp", bufs=3) as tmp_pool,
            tc.tile_pool(name="gp", bufs=8) as g_pool,
            tc.tile_pool(name="selp", bufs=4) as sel_pool,
            tc.tile_pool(name="outp", bufs=1) as out_pool,
        ):
            iota = const_pool.tile([P, ELEM], mybir.dt.int32)
            nc.gpsimd.iota(iota[:, :], pattern=[[1, ELEM]], base=0, channel_multiplier=0)

            off = const_pool.tile([P, cols], mybir.dt.int32)
            cr_t = const_pool.tile([P, cols], mybir.dt.int32)
            nc.sync.dma_start(out=cr_t[:, :], in_=cR.ap())
            # off = c & 63
            nc.vector.tensor_scalar(
                out=off[:, :],
                in0=cr_t[:, :],
                scalar1=ELEM - 1,
                scalar2=None,
                op0=mybir.AluOpType.bitwise_and,
            )

            blk16 = blk_pool.tile([P, f16], mybir.dt.int16)
            vals = out_pool.tile([P, cols], mybir.dt.float32)

            with tc.For_i(0, reps, 1):
                for lc in range(n_lchunks):
                    fsl = slice(lc * (LCHUNK // 16), (lc + 1) * (LCHUNK // 16))
                    pt = pair_pool.tile([P, LCHUNK // 16, 2], mybir.dt.int32, tag="pt")
                    nc.sync.dma_start(out=pt[:, :, :], in_=pairs.ap()[:, fsl, :])
                    # blk = r*64 + (c >> 6), in [0, 32768)
                    b32 = tmp_pool.tile([P, LCHUNK // 16], mybir.dt.int32, tag="b32")
                    nc.vector.tensor_scalar(
                        out=b32[:, :],
                        in0=pt[:, :, 1],
                        scalar1=6,
                        scalar2=None,
                        op0=mybir.AluOpType.logical_shift_right,
                    )
                    b32b = tmp_pool.tile([P, LCHUNK // 16], mybir.dt.int32, tag="b32b")
                    nc.vector.tensor_scalar(
                        out=b32b[:, :],
                        in0=pt[:, :, 0],
                        scalar1=6,
                        scalar2=None,
                        op0=mybir.AluOpType.logical_shift_left,
                    )
                    nc.vector.tensor_tensor(
                        out=b32[:, :],
                        in0=b32[:, :],
                        in1=b32b[:, :],
                        op=mybir.AluOpType.add,
                    )
                    # cast to int16 (values < 32768)
                    nc.vector.tensor_copy(out=blk16[:, fsl], in_=b32[:, :])

                    for gi in range(gathers_per_l):
                        c = lc * gathers_per_l + gi
                        gsl = slice(c * (GCHUNK // 16), (c + 1) * (GCHUNK // 16))
                        g = g_pool.tile([P, cg, ELEM], mybir.dt.float32, tag="g")
                        nc.gpsimd.dma_gather(
                            out_ap=g[:, :, :],
                            in_ap=xs_rows,
                            idxs_ap=blk16[:, gsl],
                            num_idxs=GCHUNK,
                            num_idxs_reg=GCHUNK,
                            elem_size=ELEM,
                            queue_num=c % NQ,
                        )
                        # select element off from each 64-block:
                        # M = (iota == off), val = sum(G*M)
                        osl = slice(c * cg, (c + 1) * cg)
                        m = sel_pool.tile([P, cg, ELEM], mybir.dt.float32, tag="m")
                        nc.vector.tensor_tensor(
                            out=m[:, :, :],
                            in0=iota[:, None, :].to_broadcast([P, cg, ELEM]),
                            in1=off[:, osl, None].to_broadcast([P, cg, ELEM]),
                            op=mybir.AluOpType.is_equal,
                        )
                        nc.vector.tensor_tensor(
                            out=m[:, :, :],
                            in0=m[:, :, :],
                            in1=g[:, :, :],
                            op=mybir.AluOpType.mult,
                        )
                        nc.vector.reduce_sum(
                            out=vals[:, osl], in_=m[:, :, :], axis=mybir.AxisListType.X
                        )

            nc.sync.dma_start(out=out.ap(), in_=vals[:, :])
    nc.compile()
    return nc


_NC_CACHE = {}


def _get_nc():
    if "nc" not in _NC_CACHE:
        _NC_CACHE["nc"] = build_kernel()
    return _NC_CACHE["nc"]


def _route(indices):
    """Host-side shard: route pair rows to cores by row-stripe."""
    r = indices[:, 0].astype(np.int64)
    c = indices[:, 1].astype(np.int64)
    stripe = r >> 9  # 512 rows per stripe
    order = np.argsort(stripe, kind="stable")
    counts = np.bincount(stripe, minlength=NCORES)
    assert counts.max() <= NPAD, f"stripe count {counts.max()} exceeds NPAD={NPAD}"
    in_maps = []
    gather_pos = []  # original pair-row index per core, routed order
    starts = np.concatenate([[0], np.cumsum(counts)])
    for i in range(NCORES):
        pos = order[starts[i] : starts[i + 1]]
        n = len(pos)
        rl = np.zeros(NPAD, np.int32)
        cc = np.zeros(NPAD, np.int32)
        rl[:n] = (r[pos] - i * STRIPE_ROWS).astype(np.int32)
        cc[:n] = c[pos].astype(np.int32)
        # wrapped [16, NPAD/16, 2], replicated x8 -> [128, NPAD/16, 2]
        pw = np.empty((16, NPAD // 16, 2), np.int32)
        pw[:, :, 0] = rl.reshape(NPAD // 16, 16).T
        pw[:, :, 1] = cc.reshape(NPAD // 16, 16).T
        pw = np.tile(pw, (8, 1, 1))
        # out-slot order [128, NPAD/128]
        cr = cc.reshape(NPAD // P, P).T.copy()
        in_maps.append({"pairs": pw, "cR": cr})
        gather_pos.append(pos)
    return in_maps, gather_pos


def kernel(X, indices):
    X = np.ascontiguousarray(np.asarray(X), dtype=np.float32)
    indices = np.asarray(indices, dtype=np.int32)
    nc = _get_nc()
    in_maps, gather_pos = _route(indices)
    for i in range(NCORES):
        in_maps[i]["XS"] = np.ascontiguousarray(
            X[i * STRIPE_ROWS : (i + 1) * STRIPE_ROWS]
        )
    res = run_bass_kernel_spmd(nc, in_maps, core_ids=list(range(NCORES)))
    out_flat = np.empty(N_IDX, np.float32)
    k = np.arange(NPAD)
    land = (k % P) * COLS + k // P  # routed slot k -> position in returned [P*COLS]
    for i in range(NCORES):
        vals = res.results[i]["out"].reshape(-1)
        pos = gather_pos[i]
        out_flat[pos] = vals[land[: len(pos)]]
    return out_flat.reshape(-1, 2)
